# revision 1
# baseline (speedup 1.0000x reference)
"""Trainium2 Bass kernel for nn_AttentionLayer (B=4, S=2048, D=1024, H=16).

Self-contained: builds and compiles an SPMD Bass/Tile program once, then
runs it across 8 NeuronCores via run_bass_kernel_spmd.

Sharding (no collectives): core c handles batch b = c // 2 and query-token
half c % 2 (1024 query tokens). Each core receives pre-transposed bf16
activations (x^T slices) plus bf16 weights, computes its [1024, 1024]
slice of the final layernorm output in fp32, and the host reassembles.

Per-core pipeline (all matmuls bf16 with fp32 PSUM accumulation):
- K / V projections as dense up-front TensorE phases (V in natural token-
  major layout with a per-head ones column so each head's attn@V matmul
  also produces its softmax denominator row).
- Attention processes heads sequentially: scores^T = Kh @ Qh^T into
  double-buffered PSUM, exp on ScalarE (scale=1/8 folded into the
  activation), attn@V accumulation; Q^T/residual projections are emitted
  one matmul per kb-step to fill TensorE slack inside the ACT-bound loop.
- Softmax normalization is deferred: denominators go to DRAM; per pair a
  broadcast-DMA + fast approximate reciprocal + one multiply normalizes
  the bf16 context off the critical path.
- FC + residual + layernorm finish per 128-token block.
"""

import numpy as np
import ml_dtypes


from contextlib import ExitStack

import concourse.bass as bass
import concourse.tile as tile
import concourse.mybir as mybir
from concourse import bacc

F32 = mybir.dt.float32
BF16 = mybir.dt.bfloat16
AF = mybir.ActivationFunctionType
ALU = mybir.AluOpType


def bcast_ap(ap: bass.AP, parts: int) -> bass.AP:
    """Partition-broadcast a [1, N]-shaped DRAM AP to [parts, N]."""
    return bass.AP(tensor=ap.tensor, offset=ap.offset,
                   ap=[[0, parts]] + list(ap.ap[-1:]))


def nsplits(total, cap=512):
    return [(i, min(cap, total - i)) for i in range(0, total, cap)]


def build(T=1024, S=2048, D=1024, H=16, DK=64, n_cores=8, eps=1e-5,
          trn_type="TRN2"):
    assert DK == 64 and H % 2 == 0 and D == H * DK
    DB = D // 128     # contraction chunks over d
    EB = D // 128     # e blocks (projection output chunks); == H//2
    TB = T // 128
    SB = S // 128
    PAIRS = H // 2
    VW = 65           # per-head vp stripe: 64 v columns + 1 ones column
    DEN_F = 2 * T // 128  # free size of the per-pair denominator tile

    nc = bacc.Bacc(trn_type, target_bir_lowering=False, debug=False,
                   num_devices=n_cores)

    qT = nc.dram_tensor("qT", [D, T], BF16, kind="ExternalInput").ap()
    kT = nc.dram_tensor("kT", [D, S], BF16, kind="ExternalInput").ap()
    vT = nc.dram_tensor("vT", [D, S], BF16, kind="ExternalInput").ap()
    Wq = nc.dram_tensor("Wq", [D, D], BF16, kind="ExternalInput").ap()
    Wk = nc.dram_tensor("Wk", [D, D], BF16, kind="ExternalInput").ap()
    Wv = nc.dram_tensor("Wv", [D, D], BF16, kind="ExternalInput").ap()
    Wfc = nc.dram_tensor("Wfc", [D, D], BF16, kind="ExternalInput").ap()
    bq = nc.dram_tensor("bq", [D], F32, kind="ExternalInput").ap()
    bk = nc.dram_tensor("bk", [D], F32, kind="ExternalInput").ap()
    bv = nc.dram_tensor("bv", [D], F32, kind="ExternalInput").ap()
    bfc = nc.dram_tensor("bfc", [D], F32, kind="ExternalInput").ap()
    gamma = nc.dram_tensor("gamma", [D], F32, kind="ExternalInput").ap()
    beta = nc.dram_tensor("beta", [D], F32, kind="ExternalInput").ap()
    out = nc.dram_tensor("out", [T, D], F32, kind="ExternalOutput").ap()

    den_dram = nc.dram_tensor("den_scratch", [H, T], F32).ap()
    qp_dram = nc.dram_tensor("qp_scratch", [T, D], F32).ap()
    rec_dram = nc.dram_tensor("rec_scratch", [H, T], F32).ap()

    with tile.TileContext(nc) as tc, ExitStack() as ctx:
        pconst = ctx.enter_context(tc.tile_pool(name="const", bufs=1))
        ppers = ctx.enter_context(tc.tile_pool(name="persist", bufs=1))

        # ---- tiny constants -------------------------------------------
        bqT = pconst.tile([128, EB], F32, tag="bqT", name="bqT")
        nc.sync.dma_start(out=bqT, in_=bq.rearrange("(e p) -> p e", p=128))
        bkT = pconst.tile([128, EB], F32, tag="bkT", name="bkT")
        nc.sync.dma_start(out=bkT, in_=bk.rearrange("(e p) -> p e", p=128))
        eps_t = pconst.tile([128, 1], F32, tag="eps", name="eps")
        nc.vector.memset(eps_t, eps)

        # ---- persistent outputs ---------------------------------------
        kpT_sb = [ppers.tile([128, S], BF16, tag=f"kpT{e}", name=f"kpT{e}")
                  for e in range(EB)]
        vp_sb = [ppers.tile([128, H * VW], BF16, tag=f"vp{s}", name=f"vp{s}")
                 for s in range(SB)]
        ctxT_sb = [ppers.tile([128, T], BF16, tag=f"ctxT{e}", name=f"ctxT{e}")
                   for e in range(EB)]

        pqx = ctx.enter_context(tc.tile_pool(name="qx", bufs=1))
        pwq = ctx.enter_context(tc.tile_pool(name="wq", bufs=1))
        qx_sb = [pqx.tile([128, T], BF16, tag=f"qx{d}", name=f"qx{d}")
                 for d in range(DB)]
        wq_sb = [pwq.tile([128, D], BF16, tag=f"wq{d}", name=f"wq{d}")
                 for d in range(DB)]

        # ================= K projection =================================
        # c-outer loop + per-half kx loads so the first matmul only waits
        # for wk + the first half of kT.
        with tc.tile_pool(name="wk", bufs=1) as pw, \
             tc.tile_pool(name="kx", bufs=1) as pkx, \
             tc.tile_pool(name="kps", bufs=3, space="PSUM") as pps:
            CK = min(S, 1024)
            NC_ = len(nsplits(S, CK))
            wk_sb = [pw.tile([128, D], BF16, tag=f"wk{d}", name=f"wk{d}")
                     for d in range(DB)]
            kx_sb = [[pkx.tile([128, CK], BF16, tag=f"kx{d}_{c}",
                               name=f"kx{d}_{c}") for c in range(NC_)]
                     for d in range(DB)]
            for d in range(DB):
                nc.sync.dma_start(out=wk_sb[d], in_=Wk[d * 128:(d + 1) * 128, :])
            for ci, (c0, cn) in enumerate(nsplits(S, CK)):
                for d in range(DB):
                    nc.sync.dma_start(out=kx_sb[d][ci][:, 0:cn],
                                      in_=kT[d * 128:(d + 1) * 128, c0:c0 + cn])
            # qx/wq load after kx (needed later, at attention start)
            for d in range(DB):
                nc.sync.dma_start(out=qx_sb[d], in_=qT[d * 128:(d + 1) * 128, :])
                nc.sync.dma_start(out=wq_sb[d], in_=Wq[d * 128:(d + 1) * 128, :])
            for ci, (c0, cn) in enumerate(nsplits(S, CK)):
                for e in range(EB):
                    ps = pps.tile([128, CK], F32, tag="kpT_ps", name="kpT_ps")
                    for d in range(DB):
                        for n0, nn in nsplits(cn):
                            nc.tensor.matmul(
                                ps[:, n0:n0 + nn],
                                lhsT=wk_sb[d][:, e * 128:(e + 1) * 128],
                                rhs=kx_sb[d][ci][:, n0:n0 + nn],
                                start=(d == 0), stop=(d == DB - 1))
                    nc.vector.tensor_scalar(
                        out=kpT_sb[e][:, c0:c0 + cn], in0=ps[:, 0:cn],
                        scalar1=bkT[:, e:e + 1], scalar2=None, op0=ALU.add)

        # ================= V projection (natural layout) ================
        with tc.tile_pool(name="wv", bufs=1) as pw, \
             tc.tile_pool(name="vx", bufs=1) as pvx, \
             tc.tile_pool(name="vbc", bufs=1) as pvbc, \
             tc.tile_pool(name="vps", bufs=3, space="PSUM") as pps:
            bv_bc = pvbc.tile([128, D], F32, tag="bv_bc", name="bv_bc")
            nc.gpsimd.dma_start(out=bv_bc, in_=bcast_ap(bv, 128))
            wv_sb = [pw.tile([128, D], BF16, tag=f"wv{d}", name=f"wv{d}")
                     for d in range(DB)]
            vx_sb = [pvx.tile([128, S], BF16, tag=f"vx{d}", name=f"vx{d}")
                     for d in range(DB)]
            for d in range(DB):
                nc.sync.dma_start(out=wv_sb[d], in_=Wv[d * 128:(d + 1) * 128, :])
                nc.sync.dma_start(out=vx_sb[d], in_=vT[d * 128:(d + 1) * 128, :])
            for s in range(SB):
                ps = pps.tile([128, D], F32, tag="vp_ps", name="vp_ps")
                for d in range(DB):
                    for n0, nn in nsplits(D):
                        nc.tensor.matmul(
                            ps[:, n0:n0 + nn],
                            lhsT=vx_sb[d][:, s * 128:(s + 1) * 128],
                            rhs=wv_sb[d][:, n0:n0 + nn],
                            start=(d == 0), stop=(d == DB - 1))
                vr = vp_sb[s].rearrange("p (h c) -> p h c", c=VW)
                nc.vector.tensor_add(
                    out=vr[:, :, 0:64],
                    in0=ps.rearrange("p (h c) -> p h c", c=DK),
                    in1=bv_bc.rearrange("p (h c) -> p h c", c=DK))
                nc.vector.memset(vr[:, :, 64:65], 1.0)

        # ================= attention (+ Q-proj, qp-nat interleaved) =====
        pwfc = ctx.enter_context(tc.tile_pool(name="wfc", bufs=1))
        wfc_sb = [pwfc.tile([128, D], BF16, tag=f"wfc{d}", name=f"wfc{d}")
                  for d in range(DB)]
        for d in range(DB):
            nc.gpsimd.dma_start(out=wfc_sb[d], in_=Wfc[d * 128:(d + 1) * 128, :])

        pbqfc = ctx.enter_context(tc.tile_pool(name="bqfcp", bufs=1))
        bqfc_bc = pbqfc.tile([128, D], F32, tag="bqfc", name="bqfc")
        nc.gpsimd.dma_start(out=bqfc_bc, in_=bcast_ap(bq, 128))
        tmp_bfc = pbqfc.tile([128, D], F32, tag="tmp_bfc", name="tmp_bfc")
        nc.gpsimd.dma_start(out=tmp_bfc, in_=bcast_ap(bfc, 128))
        nc.vector.tensor_add(out=bqfc_bc, in0=bqfc_bc, in1=tmp_bfc)

        with tc.tile_pool(name="scps", bufs=2, space="PSUM") as psc, \
             tc.tile_pool(name="cxps", bufs=1, space="PSUM") as pcx, \
             tc.tile_pool(name="pjps", bufs=1, space="PSUM") as ppj, \
             tc.tile_pool(name="qpT", bufs=3) as pqpt, \
             tc.tile_pool(name="attn", bufs=4) as patn, \
             tc.tile_pool(name="den", bufs=2) as pden, \
             tc.tile_pool(name="qpev", bufs=2) as pqpe, \
             tc.tile_pool(name="norm", bufs=2) as pnm, \
             tc.tile_pool(name="ctmp", bufs=2) as ptmp:
            qpT_tiles = {}

            def make_proj_thunks(jj):
                """Q^T proj + residual proj for pair jj as single-matmul
                thunks, consumed one per attention kb-step so TensorE slack
                is filled without stalling the exp stream."""
                state = {}
                thunks = []

                def qps_mm(d, n0, nn):
                    def f():
                        if 'qps' not in state:
                            state['qps'] = ppj.tile([128, T], F32, tag="pj",
                                                    name="pjq")
                        nc.tensor.matmul(
                            state['qps'][:, n0:n0 + nn],
                            lhsT=wq_sb[d][:, jj * 128:(jj + 1) * 128],
                            rhs=qx_sb[d][:, n0:n0 + nn],
                            start=(d == 0), stop=(d == DB - 1))
                    return f

                def qpt_evac():
                    qt = pqpt.tile([128, T], BF16, tag="qpT_t", name="qpT_t")
                    nc.vector.tensor_scalar(out=qt, in0=state['qps'],
                                            scalar1=bqT[:, jj:jj + 1],
                                            scalar2=None, op0=ALU.add)
                    qpT_tiles[jj] = qt

                def nps_mm(d, n0, nn):
                    def f():
                        if 'nps' not in state:
                            state['nps'] = ppj.tile([128, D], F32, tag="pj",
                                                    name="pjn")
                        nc.tensor.matmul(
                            state['nps'][:, n0:n0 + nn],
                            lhsT=qx_sb[d][:, jj * 128:(jj + 1) * 128],
                            rhs=wq_sb[d][:, n0:n0 + nn],
                            start=(d == 0), stop=(d == DB - 1))
                    return f

                def qp_evac():
                    ev = pqpe.tile([128, D], F32, tag="qp_ev", name="qp_ev")
                    nc.vector.tensor_add(out=ev, in0=state['nps'], in1=bqfc_bc)
                    nc.sync.dma_start(out=qp_dram[jj * 128:(jj + 1) * 128, :],
                                      in_=ev)

                for d in range(DB):
                    for n0, nn in nsplits(T):
                        thunks.append(qps_mm(d, n0, nn))
                thunks.append(qpt_evac)
                for d in range(DB):
                    for n0, nn in nsplits(D):
                        thunks.append(nps_mm(d, n0, nn))
                thunks.append(qp_evac)
                return thunks

            # pair 0's projections run during the V phase / attention ramp
            for th in make_proj_thunks(0):
                th()

            for j in range(PAIRS):
                qpT_t = qpT_tiles.pop(j)
                pending = make_proj_thunks(j + 1) if j + 1 < PAIRS else []
                for hh in range(2):
                    h = 2 * j + hh
                    pr = slice(hh * 64, hh * 64 + 64)
                    cx = pcx.tile([VW, T], F32, tag="cx", name="cx")
                    for kb in range(SB):
                        sc = psc.tile([128, T], F32, tag="sc", name="sc")
                        for n0, nn in nsplits(T):
                            nc.tensor.matmul(
                                sc[:, n0:n0 + nn],
                                lhsT=kpT_sb[j][pr, kb * 128:(kb + 1) * 128],
                                rhs=qpT_t[pr, n0:n0 + nn],
                                start=True, stop=True)
                        at = patn.tile([128, T], BF16, tag="at", name="at")
                        nc.scalar.activation(out=at, in_=sc, func=AF.Exp,
                                             scale=0.125)
                        vr = vp_sb[kb].rearrange("p (h c) -> p h c", c=VW)
                        for n0, nn in nsplits(T):
                            nc.tensor.matmul(
                                cx[:, n0:n0 + nn],
                                lhsT=vr[:, h, :],
                                rhs=at[:, n0:n0 + nn],
                                start=(kb == 0), stop=(kb == SB - 1))
                        if pending:
                            pending.pop(0)()
                    # evacuate unnormalized ctx + denominator
                    den = pden.tile([VW, T], F32, tag="den", name="den")
                    nc.vector.tensor_copy(out=den[64:65, :], in_=cx[64:65, :])
                    nc.gpsimd.dma_start(out=den_dram[h, :], in_=den[64:65, :])
                    if hh == 0:
                        nc.vector.tensor_copy(out=ctxT_sb[j][0:64, :],
                                              in_=cx[0:64, :])
                    else:
                        tmp = ptmp.tile([64, T], BF16, tag="ctmp", name="ctmp")
                        nc.vector.tensor_copy(out=tmp, in_=cx[0:64, :])
                        nc.sync.dma_start(out=ctxT_sb[j][64:128, :], in_=tmp)
                while pending:
                    pending.pop(0)()
                # normalize this pair's ctxT (cheap chain, off critical path)
                dbc = pnm.tile([128, T], F32, tag="dbc", name="dbc")
                nc.gpsimd.dma_start(
                    out=dbc[0:64, :],
                    in_=bcast_ap(den_dram[2 * j:2 * j + 1, :], 64))
                nc.gpsimd.dma_start(
                    out=dbc[64:128, :],
                    in_=bcast_ap(den_dram[2 * j + 1:2 * j + 2, :], 64))
                rbc = pnm.tile([128, T], F32, tag="rbc", name="rbc")
                nc.vector.reciprocal_approx_fast(out=rbc, in_=dbc)
                nc.vector.tensor_mul(out=ctxT_sb[j], in0=ctxT_sb[j], in1=rbc)

        # ================= FC + residual + layernorm ====================
        with tc.tile_pool(name="fcps", bufs=2, space="PSUM") as pfc, \
             tc.tile_pool(name="lnbc", bufs=1) as plnb, \
             tc.tile_pool(name="qpl", bufs=2) as pqp, \
             tc.tile_pool(name="xln", bufs=2) as px, \
             tc.tile_pool(name="stat", bufs=4) as pst:
            gamma_bc = plnb.tile([128, D], F32, tag="gamma_bc", name="gamma_bc")
            nc.gpsimd.dma_start(out=gamma_bc, in_=bcast_ap(gamma, 128))
            beta_bc = plnb.tile([128, D], F32, tag="beta_bc", name="beta_bc")
            nc.gpsimd.dma_start(out=beta_bc, in_=bcast_ap(beta, 128))

            for t in range(TB):
                qp_t = pqp.tile([128, D], F32, tag="qp_t", name="qp_t")
                nc.sync.dma_start(out=qp_t,
                                  in_=qp_dram[t * 128:(t + 1) * 128, :])
                fc = pfc.tile([128, D], F32, tag="fc", name="fc")
                for j in range(EB):
                    for n0, nn in nsplits(D):
                        nc.tensor.matmul(
                            fc[:, n0:n0 + nn],
                            lhsT=ctxT_sb[j][:, t * 128:(t + 1) * 128],
                            rhs=wfc_sb[j][:, n0:n0 + nn],
                            start=(j == 0), stop=(j == EB - 1))
                x = px.tile([128, D], F32, tag="x", name="x")
                nc.vector.tensor_add(out=x, in0=fc, in1=qp_t)
                ngr = max(D // 512, 1)
                gsz = min(D, 512)
                stats = pst.tile([128, ngr, 6], F32, tag="stats", name="stats")
                for g in range(ngr):
                    nc.vector.bn_stats(out=stats[:, g, :],
                                       in_=x[:, g * gsz:(g + 1) * gsz])
                mv = pst.tile([128, 2], F32, tag="mv", name="mv")
                nc.vector.bn_aggr(out=mv, in_=stats)
                rstd = pst.tile([128, 1], F32, tag="rstd", name="rstd")
                nc.scalar.activation(out=rstd, in_=mv[:, 1:2], func=AF.Sqrt,
                                     bias=eps_t, scale=1.0)
                nc.vector.reciprocal(out=rstd, in_=rstd)
                xn = px.tile([128, D], F32, tag="xn", name="xn")
                nc.vector.tensor_scalar(out=xn, in0=x, scalar1=mv[:, 0:1],
                                        scalar2=rstd, op0=ALU.subtract,
                                        op1=ALU.mult)
                nc.vector.tensor_mul(out=xn, in0=xn, in1=gamma_bc)
                nc.gpsimd.tensor_add(out=xn, in0=xn, in1=beta_bc)
                nc.sync.dma_start(out=out[t * 128:(t + 1) * 128, :], in_=xn)

    nc.compile()
    return nc


_B, _S, _D, _H, _DK = 4, 2048, 1024, 16, 64
_T = _S // 2
_NCORES = 8
_BF = ml_dtypes.bfloat16

_nc_cache = [None]


def _get_nc():
    if _nc_cache[0] is None:
        _nc_cache[0] = build(T=_T, S=_S, D=_D, H=_H, DK=_DK, n_cores=_NCORES)
    return _nc_cache[0]


def _execute(inputs, trace=False):
    from concourse.bass_utils import run_bass_kernel_spmd

    nc = _get_nc()
    q = np.asarray(inputs["q"], np.float32)
    k = np.asarray(inputs["k"], np.float32)
    v = np.asarray(inputs["v"], np.float32)
    Wq = np.asarray(inputs["Wq"], np.float32).astype(_BF)
    Wk = np.asarray(inputs["Wk"], np.float32).astype(_BF)
    Wv = np.asarray(inputs["Wv"], np.float32).astype(_BF)
    Wfc = np.asarray(inputs["Wfc"], np.float32).astype(_BF)
    fp = {n: np.asarray(inputs[n], np.float32)
          for n in ("bq", "bk", "bv", "bfc", "gamma", "beta")}

    in_maps = []
    for c in range(_NCORES):
        b, half = divmod(c, 2)
        t0 = half * _T
        in_maps.append({
            "qT": np.ascontiguousarray(q[b, t0:t0 + _T].T).astype(_BF),
            "kT": np.ascontiguousarray(k[b].T).astype(_BF),
            "vT": np.ascontiguousarray(v[b].T).astype(_BF),
            "Wq": Wq, "Wk": Wk, "Wv": Wv, "Wfc": Wfc, **fp,
        })

    res = run_bass_kernel_spmd(nc, in_maps, core_ids=list(range(_NCORES)),
                               trace=trace)
    out = np.empty((_B, _S, _D), np.float32)
    for c in range(_NCORES):
        b, half = divmod(c, 2)
        out[b, half * _T:(half + 1) * _T] = res.results[c]["out"]
    return out, res.exec_time_ns


def kernel(**inputs) -> np.ndarray:
    out, _ = _execute(inputs, trace=False)
    return out



# revision 6
# speedup vs baseline: 1.1444x; 1.1444x over previous
"""Trainium2 Bass kernel for nn_AttentionLayer (B=4, S=2048, D=1024, H=16).

Self-contained: builds and compiles an SPMD Bass/Tile program once, then
runs it across 8 NeuronCores via run_bass_kernel_spmd.

Sharding (no collectives): core c handles batch b = c // 2 and query-token
half c % 2 (1024 query tokens). Each core receives pre-transposed bf16
activations (x^T slices) plus bf16 weights, computes its [1024, 1024]
slice of the final layernorm output in fp32, and the host reassembles.

v2 pipeline (all matmuls bf16 with fp32 PSUM accumulation):
- K projection (e-outer so pair 0's keys land first), then Q^T projection
  for all pairs (the only Q projection: the residual is reconstructed
  later by identity matmuls), then V projection in natural token-major
  layout with a per-head ones column (so attn@V also yields the softmax
  denominator row).
- Attention per head pair: scores^T = Kh @ Qh^T with the two heads'
  K=64 matmuls row-tiled onto disjoint PE-array row halves (concurrent).
  exp for head A runs on ScalarE (native activation); head B's exp runs
  on VectorE as a Schraudolph bit-trick: one tensor_scalar fp32->int16
  whose result bits are exactly bf16 exp values (rel err ~3%, which
  cancels in softmax normalization). attn@V consumes the bf16 views.
- Deferred softmax normalization via DRAM-broadcast denominators +
  fast approximate reciprocal.
- FC phase: ctx matmuls + residual via identity matmuls (transposing
  Q^T tiles into the PSUM accumulation) + bfc via a K=1 ones matmul,
  then layernorm per 128-token block.
"""

import numpy as np
import ml_dtypes


from contextlib import ExitStack

import concourse.bass as bass
import concourse.tile as tile
import concourse.mybir as mybir
from concourse import bacc

F32 = mybir.dt.float32
BF16 = mybir.dt.bfloat16
I16 = mybir.dt.int16
AF = mybir.ActivationFunctionType
ALU = mybir.AluOpType

LOG2E = 1.4426950408889634
# exp(x/8) ~= bf16_bits(int16(x * SCH_A + SCH_B)) (Schraudolph, bf16 top bits)
SCH_A = float((1 << 23) * LOG2E) * 0.125 / 65536.0
SCH_B = (float(127 << 23) - 366393.0) / 65536.0
# 1/x ~= bf16_bits(RCP_MAGIC - bf16_bits(x)), x > 0 (max rel err ~5%)
RCP_MAGIC = 0x7EF3


def bcast_ap(ap: bass.AP, parts: int) -> bass.AP:
    """Partition-broadcast a [1, N]-shaped DRAM AP to [parts, N]."""
    return bass.AP(tensor=ap.tensor, offset=ap.offset,
                   ap=[[0, parts]] + list(ap.ap[-1:]))


def nsplits(total, cap=512):
    return [(i, min(cap, total - i)) for i in range(0, total, cap)]


def build(T=1024, S=2048, D=1024, H=16, DK=64, n_cores=8, eps=1e-5,
          trn_type="TRN2"):
    assert DK == 64 and H % 2 == 0 and D == H * DK
    DB = D // 128     # contraction chunks over d
    EB = D // 128     # e blocks (projection output chunks); == H//2
    TB = T // 128
    SB = S // 128
    PAIRS = H // 2
    VW = 65           # per-head vp stripe: 64 v columns + 1 ones column

    nc = bacc.Bacc(trn_type, target_bir_lowering=False, debug=False,
                   num_devices=n_cores)

    qT = nc.dram_tensor("qT", [D, T], BF16, kind="ExternalInput").ap()
    kT = nc.dram_tensor("kT", [D, S], BF16, kind="ExternalInput").ap()
    vT = nc.dram_tensor("vT", [D, S], BF16, kind="ExternalInput").ap()
    Wq = nc.dram_tensor("Wq", [D, D], BF16, kind="ExternalInput").ap()
    Wk = nc.dram_tensor("Wk", [D, D], BF16, kind="ExternalInput").ap()
    Wv = nc.dram_tensor("Wv", [D, D], BF16, kind="ExternalInput").ap()
    Wfc = nc.dram_tensor("Wfc", [D, D], BF16, kind="ExternalInput").ap()
    bq = nc.dram_tensor("bq", [D], F32, kind="ExternalInput").ap()
    bk = nc.dram_tensor("bk", [D], F32, kind="ExternalInput").ap()
    bv = nc.dram_tensor("bv", [D], F32, kind="ExternalInput").ap()
    bfch = nc.dram_tensor("bfch", [D], BF16, kind="ExternalInput").ap()
    gamma = nc.dram_tensor("gamma", [D], F32, kind="ExternalInput").ap()
    beta = nc.dram_tensor("beta", [D], F32, kind="ExternalInput").ap()
    ident = nc.dram_tensor("ident", [128, 128], BF16, kind="ExternalInput").ap()
    out = nc.dram_tensor("out", [T, D], F32, kind="ExternalOutput").ap()

    den_dram = nc.dram_tensor("den_scratch", [H, T], BF16).ap()

    with tile.TileContext(nc) as tc, ExitStack() as ctx:
        pconst = ctx.enter_context(tc.tile_pool(name="const", bufs=1))
        ppers = ctx.enter_context(tc.tile_pool(name="persist", bufs=1))

        # ---- tiny constants -------------------------------------------
        bqT = pconst.tile([128, EB], F32, tag="bqT", name="bqT")
        nc.sync.dma_start(out=bqT, in_=bq.rearrange("(e p) -> p e", p=128))
        bkT = pconst.tile([128, EB], F32, tag="bkT", name="bkT")
        nc.sync.dma_start(out=bkT, in_=bk.rearrange("(e p) -> p e", p=128))
        eps_t = pconst.tile([128, 1], F32, tag="eps", name="eps")
        nc.vector.memset(eps_t, eps)
        i_sb = pconst.tile([128, 128], BF16, tag="ident", name="ident")
        nc.sync.dma_start(out=i_sb, in_=ident)
        ones1 = pconst.tile([1, 128], BF16, tag="ones1", name="ones1")
        nc.vector.memset(ones1, 1.0)
        bfc_sb = pconst.tile([1, D], BF16, tag="bfc_sb", name="bfc_sb")
        nc.sync.dma_start(out=bfc_sb, in_=bcast_ap(bfch, 1))

        # ---- persistent tiles -----------------------------------------
        kpT_sb = [ppers.tile([128, S], BF16, tag=f"kpT{e}", name=f"kpT{e}")
                  for e in range(EB)]
        vp_sb = [ppers.tile([128, H * VW], BF16, tag=f"vp{s}", name=f"vp{s}")
                 for s in range(SB)]
        ctxT_sb = [ppers.tile([128, T], BF16, tag=f"ctxT{e}", name=f"ctxT{e}")
                   for e in range(EB)]
        qpT_sb = [ppers.tile([128, T], BF16, tag=f"qpT{j}", name=f"qpT{j}")
                  for j in range(PAIRS)]

        # ================= K projection (e-outer) ======================
        with tc.tile_pool(name="wk", bufs=1) as pw, \
             tc.tile_pool(name="kx", bufs=1) as pkx, \
             tc.tile_pool(name="qx", bufs=1) as pqx, \
             tc.tile_pool(name="wq", bufs=1) as pwq, \
             tc.tile_pool(name="kps", bufs=2, space="PSUM") as pps, \
             tc.tile_pool(name="qps", bufs=2, space="PSUM") as pqps:
            CK = min(S, 1024)
            NC_ = len(nsplits(S, CK))
            wk_sb = [pw.tile([128, D], BF16, tag=f"wk{d}", name=f"wk{d}")
                     for d in range(DB)]
            kx_sb = [[pkx.tile([128, CK], BF16, tag=f"kx{d}_{c}",
                               name=f"kx{d}_{c}") for c in range(NC_)]
                     for d in range(DB)]
            for d in range(DB):
                nc.sync.dma_start(out=wk_sb[d], in_=Wk[d * 128:(d + 1) * 128, :])
            for ci, (c0, cn) in enumerate(nsplits(S, CK)):
                for d in range(DB):
                    nc.sync.dma_start(out=kx_sb[d][ci][:, 0:cn],
                                      in_=kT[d * 128:(d + 1) * 128, c0:c0 + cn])
            qx_sb = [pqx.tile([128, T], BF16, tag=f"qx{d}", name=f"qx{d}")
                     for d in range(DB)]
            wq_sb = [pwq.tile([128, D], BF16, tag=f"wq{d}", name=f"wq{d}")
                     for d in range(DB)]
            for d in range(DB):
                nc.sync.dma_start(out=qx_sb[d], in_=qT[d * 128:(d + 1) * 128, :])
                nc.sync.dma_start(out=wq_sb[d], in_=Wq[d * 128:(d + 1) * 128, :])
            for e in range(EB):
                for ci, (c0, cn) in enumerate(nsplits(S, CK)):
                    ps = pps.tile([128, CK], F32, tag="kpT_ps", name="kpT_ps")
                    for d in range(DB):
                        for n0, nn in nsplits(cn):
                            nc.tensor.matmul(
                                ps[:, n0:n0 + nn],
                                lhsT=wk_sb[d][:, e * 128:(e + 1) * 128],
                                rhs=kx_sb[d][ci][:, n0:n0 + nn],
                                start=(d == 0), stop=(d == DB - 1))
                    nc.vector.tensor_scalar(
                        out=kpT_sb[e][:, c0:c0 + cn], in0=ps[:, 0:cn],
                        scalar1=bkT[:, e:e + 1], scalar2=None, op0=ALU.add)

            # ============= Q^T projection (all pairs) ==================
            for j in range(PAIRS):
                qps = pqps.tile([128, T], F32, tag="qps", name="qps")
                for d in range(DB):
                    for n0, nn in nsplits(T):
                        nc.tensor.matmul(
                            qps[:, n0:n0 + nn],
                            lhsT=wq_sb[d][:, j * 128:(j + 1) * 128],
                            rhs=qx_sb[d][:, n0:n0 + nn],
                            start=(d == 0), stop=(d == DB - 1))
                nc.vector.tensor_scalar(
                    out=qpT_sb[j], in0=qps, scalar1=bqT[:, j:j + 1],
                    scalar2=None, op0=ALU.add)

        # ================= V projection (natural layout) ================
        with tc.tile_pool(name="wv", bufs=1) as pw, \
             tc.tile_pool(name="vx", bufs=1) as pvx, \
             tc.tile_pool(name="vbc", bufs=1) as pvbc, \
             tc.tile_pool(name="vps", bufs=3, space="PSUM") as pps:
            bv_bc = pvbc.tile([128, D], F32, tag="bv_bc", name="bv_bc")
            nc.gpsimd.dma_start(out=bv_bc, in_=bcast_ap(bv, 128))
            wv_sb = [pw.tile([128, D], BF16, tag=f"wv{d}", name=f"wv{d}")
                     for d in range(DB)]
            vx_sb = [pvx.tile([128, S], BF16, tag=f"vx{d}", name=f"vx{d}")
                     for d in range(DB)]
            for d in range(DB):
                nc.sync.dma_start(out=wv_sb[d], in_=Wv[d * 128:(d + 1) * 128, :])
                nc.sync.dma_start(out=vx_sb[d], in_=vT[d * 128:(d + 1) * 128, :])
            for s in range(SB):
                ps = pps.tile([128, D], F32, tag="vp_ps", name="vp_ps")
                for d in range(DB):
                    for n0, nn in nsplits(D):
                        nc.tensor.matmul(
                            ps[:, n0:n0 + nn],
                            lhsT=vx_sb[d][:, s * 128:(s + 1) * 128],
                            rhs=wv_sb[d][:, n0:n0 + nn],
                            start=(d == 0), stop=(d == DB - 1))
                vr = vp_sb[s].rearrange("p (h c) -> p h c", c=VW)
                nc.vector.tensor_add(
                    out=vr[:, :, 0:64],
                    in0=ps.rearrange("p (h c) -> p h c", c=DK),
                    in1=bv_bc.rearrange("p (h c) -> p h c", c=DK))
                nc.vector.memset(vr[:, :, 64:65], 1.0)

        # ================= attention ====================================
        pwfc = ctx.enter_context(tc.tile_pool(name="wfc", bufs=1))
        wfc_sb = [pwfc.tile([128, D], BF16, tag=f"wfc{d}", name=f"wfc{d}")
                  for d in range(DB)]
        for d in range(DB):
            nc.gpsimd.dma_start(out=wfc_sb[d], in_=Wfc[d * 128:(d + 1) * 128, :])

        with tc.tile_pool(name="scA0", bufs=1, space="PSUM") as psa0, \
             tc.tile_pool(name="scA1", bufs=1, space="PSUM") as psa1, \
             tc.tile_pool(name="scB", bufs=1, space="PSUM") as psb, \
             tc.tile_pool(name="cxps", bufs=1, space="PSUM") as pcx, \
             tc.tile_pool(name="atA", bufs=2) as pata, \
             tc.tile_pool(name="atB", bufs=2) as patb, \
             tc.tile_pool(name="den", bufs=2) as pden, \
             tc.tile_pool(name="norm", bufs=2) as pnm, \
             tc.tile_pool(name="ctmp", bufs=2) as ptmp:
            for j in range(PAIRS):
                cxa = pcx.tile([VW, T], F32, tag="cxA", name="cxA")
                cxb = pcx.tile([VW, T], F32, tag="cxB", name="cxB")
                kA = kpT_sb[j][0:64, :]
                kB = kpT_sb[j][64:128, :]
                qA = qpT_sb[j][0:64, :]
                qB = qpT_sb[j][64:128, :]
                for kb in range(SB):
                    kblk = slice(kb * 128, (kb + 1) * 128)
                    sA0 = psa0.tile([128, 512], F32, tag="sA0", name="sA0")
                    sA1 = psa1.tile([128, 512], F32, tag="sA1", name="sA1")
                    sB = psb.tile([128, T], F32, tag="sB", name="sB")
                    # row-tiled score pairs (A rows 0:64, B rows 64:128)
                    nc.tensor.matmul(sA0, lhsT=kA[:, kblk], rhs=qA[:, 0:512],
                                     start=True, stop=True)
                    nc.tensor.matmul(sB[:, 0:512], lhsT=kB[:, kblk],
                                     rhs=qB[:, 0:512], start=True, stop=True)
                    nc.tensor.matmul(sA1, lhsT=kA[:, kblk], rhs=qA[:, 512:T],
                                     start=True, stop=True)
                    nc.tensor.matmul(sB[:, 512:T], lhsT=kB[:, kblk],
                                     rhs=qB[:, 512:T], start=True, stop=True)
                    # exp: head A on ScalarE, head B on VectorE (bit trick)
                    atA = pata.tile([128, T], BF16, tag="atA", name="atA")
                    nc.scalar.activation(out=atA[:, 0:512], in_=sA0,
                                         func=AF.Exp, scale=0.125)
                    nc.scalar.activation(out=atA[:, 512:T], in_=sA1,
                                         func=AF.Exp, scale=0.125)
                    atBi = patb.tile([128, T], I16, tag="atB", name="atB")
                    nc.vector.tensor_scalar(out=atBi, in0=sB, scalar1=SCH_A,
                                            scalar2=SCH_B, op0=ALU.mult,
                                            op1=ALU.add)
                    atB = atBi.bitcast(BF16)
                    # attn@V accumulation
                    vr = vp_sb[kb].rearrange("p (h c) -> p h c", c=VW)
                    st, sp = (kb == 0), (kb == SB - 1)
                    nc.tensor.matmul(cxa[:, 0:512], lhsT=vr[:, 2 * j, :],
                                     rhs=atA[:, 0:512], start=st, stop=sp)
                    nc.tensor.matmul(cxb[:, 0:512], lhsT=vr[:, 2 * j + 1, :],
                                     rhs=atB[:, 0:512], start=st, stop=sp)
                    nc.tensor.matmul(cxa[:, 512:T], lhsT=vr[:, 2 * j, :],
                                     rhs=atA[:, 512:T], start=st, stop=sp)
                    nc.tensor.matmul(cxb[:, 512:T], lhsT=vr[:, 2 * j + 1, :],
                                     rhs=atB[:, 512:T], start=st, stop=sp)
                # evacuate ctx + denominators: head A via DVE, head B via
                # ScalarE (PSUM-capable engines); den rows ride along in
                # the [65, T] staging copies, then DMA splits them out.
                stga = ptmp.tile([VW, T], BF16, tag="stga", name="stga")
                nc.vector.tensor_copy(out=stga, in_=cxa)
                stgb = ptmp.tile([VW, T], BF16, tag="stgb", name="stgb")
                nc.scalar.activation(out=stgb, in_=cxb, func=AF.Copy)
                nc.sync.dma_start(out=ctxT_sb[j][0:64, :], in_=stga[0:64, :])
                nc.sync.dma_start(out=ctxT_sb[j][64:128, :],
                                  in_=stgb[0:64, :])
                nc.gpsimd.dma_start(out=den_dram[2 * j, :],
                                    in_=stga[64:65, :])
                nc.gpsimd.dma_start(out=den_dram[2 * j + 1, :],
                                    in_=stgb[64:65, :])
                # deferred softmax normalization (off critical path):
                # magic-number bf16 reciprocal of broadcast denominators
                dbc = pnm.tile([128, T], BF16, tag="dbc", name="dbc")
                nc.gpsimd.dma_start(
                    out=dbc[0:64, :],
                    in_=bcast_ap(den_dram[2 * j:2 * j + 1, :], 64))
                nc.gpsimd.dma_start(
                    out=dbc[64:128, :],
                    in_=bcast_ap(den_dram[2 * j + 1:2 * j + 2, :], 64))
                rbc = pnm.tile([128, T], I16, tag="rbc", name="rbc")
                nc.vector.tensor_scalar(out=rbc, in0=dbc.bitcast(I16),
                                        scalar1=-1, scalar2=RCP_MAGIC,
                                        op0=ALU.mult, op1=ALU.add)
                nc.vector.tensor_mul(out=ctxT_sb[j], in0=ctxT_sb[j],
                                     in1=rbc.bitcast(BF16))

        # ================= FC + residual + layernorm ====================
        with tc.tile_pool(name="fcps", bufs=2, space="PSUM") as pfc, \
             tc.tile_pool(name="lnbc", bufs=1) as plnb, \
             tc.tile_pool(name="xln", bufs=2) as px, \
             tc.tile_pool(name="stat", bufs=4) as pst:
            gamma_bc = plnb.tile([128, D], F32, tag="gamma_bc", name="gamma_bc")
            nc.gpsimd.dma_start(out=gamma_bc, in_=bcast_ap(gamma, 128))
            beta_bc = plnb.tile([128, D], F32, tag="beta_bc", name="beta_bc")
            nc.gpsimd.dma_start(out=beta_bc, in_=bcast_ap(beta, 128))

            for t in range(TB):
                tblk = slice(t * 128, (t + 1) * 128)
                fc = pfc.tile([128, D], F32, tag="fc", name="fc")
                for c0, cn in nsplits(D):
                    for jj in range(EB):
                        nc.tensor.matmul(
                            fc[:, c0:c0 + cn],
                            lhsT=ctxT_sb[jj][:, tblk],
                            rhs=wfc_sb[jj][:, c0:c0 + cn],
                            start=(jj == 0), stop=False)
                    # residual: transpose qpT pair blocks via identity
                    for jj in range(c0 // 128, (c0 + cn) // 128):
                        nc.tensor.matmul(
                            fc[:, jj * 128:(jj + 1) * 128],
                            lhsT=qpT_sb[jj][:, tblk], rhs=i_sb,
                            start=False, stop=False)
                    # bfc bias via K=1 ones matmul (marks group end)
                    nc.tensor.matmul(
                        fc[:, c0:c0 + cn], lhsT=ones1,
                        rhs=bfc_sb[0:1, c0:c0 + cn], start=False, stop=True)
                ngr = max(D // 512, 1)
                gsz = min(D, 512)
                stats = pst.tile([128, ngr, 6], F32, tag="stats", name="stats")
                for g in range(ngr):
                    nc.vector.bn_stats(out=stats[:, g, :],
                                       in_=fc[:, g * gsz:(g + 1) * gsz])
                mv = pst.tile([128, 2], F32, tag="mv", name="mv")
                nc.vector.bn_aggr(out=mv, in_=stats)
                rstd = pst.tile([128, 1], F32, tag="rstd", name="rstd")
                nc.scalar.activation(out=rstd, in_=mv[:, 1:2], func=AF.Sqrt,
                                     bias=eps_t, scale=1.0)
                nc.vector.reciprocal(out=rstd, in_=rstd)
                xn = px.tile([128, D], F32, tag="xn", name="xn")
                nc.vector.tensor_scalar(out=xn, in0=fc, scalar1=mv[:, 0:1],
                                        scalar2=rstd, op0=ALU.subtract,
                                        op1=ALU.mult)
                xg = px.tile([128, D], F32, tag="xg", name="xg")
                nc.vector.tensor_mul(out=xg, in0=xn, in1=gamma_bc)
                nc.gpsimd.tensor_add(out=xg, in0=xg, in1=beta_bc)
                nc.sync.dma_start(out=out[tblk, :], in_=xg)

    nc.compile()
    return nc


_B, _S, _D, _H, _DK = 4, 2048, 1024, 16, 64
_T = _S // 2
_NCORES = 8
_BF = ml_dtypes.bfloat16

_nc_cache = [None]


def _get_nc():
    if _nc_cache[0] is None:
        _nc_cache[0] = build(T=_T, S=_S, D=_D, H=_H, DK=_DK, n_cores=_NCORES)
    return _nc_cache[0]


def _execute(inputs, trace=False):
    from concourse.bass_utils import run_bass_kernel_spmd

    nc = _get_nc()
    q = np.asarray(inputs["q"], np.float32)
    k = np.asarray(inputs["k"], np.float32)
    v = np.asarray(inputs["v"], np.float32)
    Wq = np.asarray(inputs["Wq"], np.float32).astype(_BF)
    Wk = np.asarray(inputs["Wk"], np.float32).astype(_BF)
    Wv = np.asarray(inputs["Wv"], np.float32).astype(_BF)
    Wfc = np.asarray(inputs["Wfc"], np.float32).astype(_BF)
    fp = {n: np.asarray(inputs[n], np.float32)
          for n in ("bq", "bk", "bv", "gamma", "beta")}
    bfch = np.asarray(inputs["bfc"], np.float32).astype(_BF)
    ident = np.eye(128, dtype=np.float32).astype(_BF)

    in_maps = []
    for c in range(_NCORES):
        b, half = divmod(c, 2)
        t0 = half * _T
        in_maps.append({
            "qT": np.ascontiguousarray(q[b, t0:t0 + _T].T).astype(_BF),
            "kT": np.ascontiguousarray(k[b].T).astype(_BF),
            "vT": np.ascontiguousarray(v[b].T).astype(_BF),
            "Wq": Wq, "Wk": Wk, "Wv": Wv, "Wfc": Wfc,
            "bfch": bfch, "ident": ident, **fp,
        })

    res = run_bass_kernel_spmd(nc, in_maps, core_ids=list(range(_NCORES)),
                               trace=trace)
    out = np.empty((_B, _S, _D), np.float32)
    for c in range(_NCORES):
        b, half = divmod(c, 2)
        out[b, half * _T:(half + 1) * _T] = res.results[c]["out"]
    return out, res.exec_time_ns


def kernel(**inputs) -> np.ndarray:
    out, _ = _execute(inputs, trace=False)
    return out


# revision 9
# speedup vs baseline: 1.2806x; 1.1190x over previous
"""Trainium2 Bass kernel for nn_AttentionLayer (B=4, S=2048, D=1024, H=16).

Self-contained: builds and compiles an SPMD Bass/Tile program once, then
runs it across 8 NeuronCores via run_bass_kernel_spmd.

Sharding (no collectives): core c handles batch b = c // 2 and query-token
half c % 2 (1024 query tokens). Each core receives pre-transposed fp8
activations (x^T slices) plus fp8/bf16 weights, computes its [1024, 1024]
slice of the final layernorm output in fp32, and the host reassembles.

v3 pipeline:
- K/Q^T/V projections and the FC context matmuls run in fp8 (e4m3) with
  DoubleRow perf mode: operands are staged as [128, 2, N] tiles holding
  two 128-row contraction slabs, halving matmul streaming time. Weights
  are host-prescaled by 64 (fp8 range); evacuations rescale by 1/64.
- Attention per head pair: scores^T = Kh @ Qh^T in bf16 with both heads'
  K=64 matmuls packed into one PSUM tile per query chunk — shared WAR
  deps make the scheduler emit them adjacently, so they row-tile onto
  disjoint PE-array halves and run concurrently.
- exp head A on ScalarE (native, bf16 out); head B on VectorE via a
  Schraudolph bit trick (one tensor_scalar fp32->int16 whose bits are
  bf16 exp values; the ~3% sawtooth error cancels in softmax).
- attn@V in bf16 with a per-head ones column producing denominators.
- Softmax normalization deferred: denominators roundtrip DRAM (bf16),
  reciprocal via a magic-number bit trick fused with the x64 fp8 scale,
  producing normalized fp8 context tiles in DoubleRow pair layout.
- FC: fp8 DR ctx matmuls + residual via identity matmuls (transposing
  Q^T tiles, identity prescaled by 4096 to match the fp8 scales) + bfc
  via a K=1 ones matmul; layernorm absorbs the 4096 scale (eps * 4096^2).
"""

import numpy as np
import ml_dtypes


from contextlib import ExitStack

import concourse.bass as bass
import concourse.tile as tile
import concourse.mybir as mybir
from concourse import bacc

F32 = mybir.dt.float32
BF16 = mybir.dt.bfloat16
I16 = mybir.dt.int16
F8 = mybir.dt.float8e4
DR = mybir.MatmulPerfMode.DoubleRow
AF = mybir.ActivationFunctionType
ALU = mybir.AluOpType

LOG2E = 1.4426950408889634
# exp(x/8) ~= bf16_bits(int16(x * SCH_A + SCH_B)) (Schraudolph, bf16 top bits)
SCH_A = float((1 << 23) * LOG2E) * 0.125 / 65536.0
SCH_B = (float(127 << 23) - 366393.0) / 65536.0
# 1/x ~= bf16_bits(RCP_MAGIC - bf16_bits(x)), x > 0 (max rel err ~5%)
RCP_MAGIC = 0x7EF3
WSCALE = 64.0          # host fp8 weight prescale
LAM = WSCALE * WSCALE  # fc psum scale (ctx*64 @ Wfc*64); LN is scale-invariant


def bcast_ap(ap: bass.AP, parts: int) -> bass.AP:
    """Partition-broadcast a [1, N]-shaped DRAM AP to [parts, N]."""
    return bass.AP(tensor=ap.tensor, offset=ap.offset,
                   ap=[[0, parts]] + list(ap.ap[-1:]))


def nsplits(total, cap=512):
    return [(i, min(cap, total - i)) for i in range(0, total, cap)]


def build(T=1024, S=2048, D=1024, H=16, DK=64, n_cores=8, eps=1e-5,
          trn_type="TRN2"):
    assert DK == 64 and H % 2 == 0 and D == H * DK
    G = D // 256      # DoubleRow contraction groups (256 rows each)
    EB = D // 128     # e blocks (projection output chunks); == H//2
    TB = T // 128
    SB = S // 128
    PAIRS = H // 2
    VW = 65           # per-head vp stripe: 64 v columns + 1 ones column

    nc = bacc.Bacc(trn_type, target_bir_lowering=False, debug=False,
                   num_devices=n_cores)

    qT = nc.dram_tensor("qT", [D, T], BF16, kind="ExternalInput").ap()
    kT = nc.dram_tensor("kT", [D, S], F8, kind="ExternalInput").ap()
    vT = nc.dram_tensor("vT", [D, S], F8, kind="ExternalInput").ap()
    Wq = nc.dram_tensor("Wq", [D, D], BF16, kind="ExternalInput").ap()
    Wk = nc.dram_tensor("Wk", [D, D], F8, kind="ExternalInput").ap()
    Wv = nc.dram_tensor("Wv", [D, D], F8, kind="ExternalInput").ap()
    Wfc = nc.dram_tensor("Wfc", [D, D], F8, kind="ExternalInput").ap()
    bq = nc.dram_tensor("bq", [D], F32, kind="ExternalInput").ap()
    bk = nc.dram_tensor("bk", [D], F32, kind="ExternalInput").ap()
    bv = nc.dram_tensor("bv", [D], F32, kind="ExternalInput").ap()
    bfch = nc.dram_tensor("bfch", [D], BF16, kind="ExternalInput").ap()
    gamma = nc.dram_tensor("gamma", [D], F32, kind="ExternalInput").ap()
    beta = nc.dram_tensor("beta", [D], F32, kind="ExternalInput").ap()
    ident = nc.dram_tensor("ident", [128, 128], BF16, kind="ExternalInput").ap()
    out = nc.dram_tensor("out", [T, D], F32, kind="ExternalOutput").ap()

    den_dram = nc.dram_tensor("den_scratch", [H, T], BF16).ap()

    def load_dr(pool, src, n, tagp):
        """Load fp8 [D, n] DRAM tensor into G [128, 2, n] DoubleRow tiles."""
        tiles = []
        for g in range(G):
            t = pool.tile([128, 2, n], F8, tag=f"{tagp}{g}", name=f"{tagp}{g}")
            nc.sync.dma_start(out=t[:, 0, :], in_=src[g * 256:g * 256 + 128, :])
            nc.sync.dma_start(out=t[:, 1, :],
                              in_=src[g * 256 + 128:g * 256 + 256, :])
            tiles.append(t)
        return tiles

    with tile.TileContext(nc) as tc, ExitStack() as ctx:
        pconst = ctx.enter_context(tc.tile_pool(name="const", bufs=1))
        ppers = ctx.enter_context(tc.tile_pool(name="persist", bufs=1))

        # ---- tiny constants -------------------------------------------
        bqT = pconst.tile([128, EB], F32, tag="bqT", name="bqT")
        nc.sync.dma_start(out=bqT, in_=bq.rearrange("(e p) -> p e", p=128))
        bkT = pconst.tile([128, EB], F32, tag="bkT", name="bkT")
        nc.sync.dma_start(out=bkT, in_=bk.rearrange("(e p) -> p e", p=128))
        eps_t = pconst.tile([128, 1], F32, tag="eps", name="eps")
        nc.vector.memset(eps_t, eps * LAM * LAM)
        i_sb = pconst.tile([128, 128], BF16, tag="ident", name="ident")
        nc.sync.dma_start(out=i_sb, in_=ident)
        ones1 = pconst.tile([1, 128], BF16, tag="ones1", name="ones1")
        nc.vector.memset(ones1, 1.0)
        bfc_sb = pconst.tile([1, D], BF16, tag="bfc_sb", name="bfc_sb")
        nc.sync.dma_start(out=bfc_sb, in_=bcast_ap(bfch, 1))

        # ---- persistent tiles -----------------------------------------
        kpT_sb = [ppers.tile([128, S], BF16, tag=f"kpT{e}", name=f"kpT{e}")
                  for e in range(EB)]
        vp_sb = [ppers.tile([128, H * VW], BF16, tag=f"vp{s}", name=f"vp{s}")
                 for s in range(SB)]
        ctxT_sb = [ppers.tile([128, T], BF16, tag=f"ctxT{e}", name=f"ctxT{e}")
                   for e in range(EB)]
        ctx8_sb = [ppers.tile([128, 2, T], F8, tag=f"ctx8_{g}",
                              name=f"ctx8_{g}") for g in range(G)]
        qpT_sb = [ppers.tile([128, T], BF16, tag=f"qpT{j}", name=f"qpT{j}")
                  for j in range(PAIRS)]

        # ================= K projection (e-outer, fp8 DR) ==============
        with tc.tile_pool(name="wk", bufs=1) as pw, \
             tc.tile_pool(name="kx", bufs=1) as pkx, \
             tc.tile_pool(name="qx", bufs=1) as pqx, \
             tc.tile_pool(name="wq", bufs=1) as pwq, \
             tc.tile_pool(name="kps", bufs=2, space="PSUM") as pps, \
             tc.tile_pool(name="qps", bufs=2, space="PSUM") as pqps:
            wk_dr = load_dr(pw, Wk, D, "wk")
            kx_dr = load_dr(pkx, kT, S, "kx")
            DB = D // 128
            qx_sb = [pqx.tile([128, T], BF16, tag=f"qx{d}", name=f"qx{d}")
                     for d in range(DB)]
            wq_sb = [pwq.tile([128, D], BF16, tag=f"wq{d}", name=f"wq{d}")
                     for d in range(DB)]
            for d in range(DB):
                nc.sync.dma_start(out=qx_sb[d], in_=qT[d * 128:(d + 1) * 128, :])
                nc.sync.dma_start(out=wq_sb[d], in_=Wq[d * 128:(d + 1) * 128, :])
            CK = min(S, 1024)
            for e in range(EB):
                ecol = slice(e * 128, (e + 1) * 128)
                for c0, cn in nsplits(S, CK):
                    ps = pps.tile([128, CK], F32, tag="kpT_ps", name="kpT_ps")
                    for g in range(G):
                        for n0, nn in nsplits(cn):
                            nc.tensor.matmul(
                                ps[:, n0:n0 + nn],
                                lhsT=wk_dr[g][:, :, ecol],
                                rhs=kx_dr[g][:, :, c0 + n0:c0 + n0 + nn],
                                start=(g == 0), stop=(g == G - 1),
                                perf_mode=DR)
                    nc.vector.tensor_scalar(
                        out=kpT_sb[e][:, c0:c0 + cn], in0=ps[:, 0:cn],
                        scalar1=1.0 / WSCALE, scalar2=bkT[:, e:e + 1],
                        op0=ALU.mult, op1=ALU.add)

            # ============= Q^T projection (all pairs, fp8 DR) ==========
            for j in range(PAIRS):
                qps = pqps.tile([128, T], F32, tag="qps", name="qps")
                for d in range(DB):
                    for n0, nn in nsplits(T):
                        nc.tensor.matmul(
                            qps[:, n0:n0 + nn],
                            lhsT=wq_sb[d][:, j * 128:(j + 1) * 128],
                            rhs=qx_sb[d][:, n0:n0 + nn],
                            start=(d == 0), stop=(d == DB - 1))
                nc.vector.tensor_scalar(
                    out=qpT_sb[j], in0=qps, scalar1=bqT[:, j:j + 1],
                    scalar2=None, op0=ALU.add)

        # ================= V projection (natural layout, fp8 DR) ========
        with tc.tile_pool(name="wv", bufs=1) as pw, \
             tc.tile_pool(name="vx", bufs=1) as pvx, \
             tc.tile_pool(name="vbc", bufs=1) as pvbc, \
             tc.tile_pool(name="vps", bufs=3, space="PSUM") as pps:
            bv_bc = pvbc.tile([128, D], F32, tag="bv_bc", name="bv_bc")
            nc.gpsimd.dma_start(out=bv_bc, in_=bcast_ap(bv, 128))
            wv_dr = load_dr(pw, Wv, D, "wv")
            vx_dr = load_dr(pvx, vT, S, "vx")
            for s in range(SB):
                ps = pps.tile([128, D], F32, tag="vp_ps", name="vp_ps")
                for g in range(G):
                    for n0, nn in nsplits(D):
                        nc.tensor.matmul(
                            ps[:, n0:n0 + nn],
                            lhsT=vx_dr[g][:, :, s * 128:(s + 1) * 128],
                            rhs=wv_dr[g][:, :, n0:n0 + nn],
                            start=(g == 0), stop=(g == G - 1), perf_mode=DR)
                vr = vp_sb[s].rearrange("p (h c) -> p h c", c=VW)
                nc.vector.scalar_tensor_tensor(
                    out=vr[:, :, 0:64],
                    in0=ps.rearrange("p (h c) -> p h c", c=DK),
                    scalar=1.0 / WSCALE,
                    in1=bv_bc.rearrange("p (h c) -> p h c", c=DK),
                    op0=ALU.mult, op1=ALU.add)
                nc.vector.memset(vr[:, :, 64:65], 1.0)

        # ================= attention ====================================
        pwfc = ctx.enter_context(tc.tile_pool(name="wfc", bufs=1))
        wfc_dr = []
        for g in range(G):
            t = pwfc.tile([128, 2, D], F8, tag=f"wfc{g}", name=f"wfc{g}")
            nc.gpsimd.dma_start(out=t[:, 0, :],
                                in_=Wfc[g * 256:g * 256 + 128, :])
            nc.gpsimd.dma_start(out=t[:, 1, :],
                                in_=Wfc[g * 256 + 128:g * 256 + 256, :])
            wfc_dr.append(t)

        with tc.tile_pool(name="scAB0", bufs=1, space="PSUM") as psc0, \
             tc.tile_pool(name="scAB1", bufs=1, space="PSUM") as psc1, \
             tc.tile_pool(name="cxps", bufs=1, space="PSUM") as pcx, \
             tc.tile_pool(name="atA", bufs=2) as pata, \
             tc.tile_pool(name="atB", bufs=2) as patb, \
             tc.tile_pool(name="norm", bufs=2) as pnm, \
             tc.tile_pool(name="ctmp", bufs=2) as ptmp:
            for j in range(PAIRS):
                cxa = pcx.tile([VW, T], F32, tag="cxA", name="cxA")
                cxb = pcx.tile([VW, T], F32, tag="cxB", name="cxB")
                kA = kpT_sb[j][0:64, :]
                kB = kpT_sb[j][64:128, :]
                qA = qpT_sb[j][0:64, :]
                qB = qpT_sb[j][64:128, :]
                for kb in range(SB):
                    kblk = slice(kb * 128, (kb + 1) * 128)
                    # both heads' scores for a query chunk share one PSUM
                    # tile (A in cols 0:512, B in 512:1024) so their WAR
                    # deps coincide -> scheduler emits them adjacently ->
                    # K=64 row-tiles run concurrently on the PE array.
                    sc0 = psc0.tile([128, T], F32, tag="sc0", name="sc0")
                    sc1 = psc1.tile([128, T], F32, tag="sc1", name="sc1")
                    nc.tensor.matmul(sc0[:, 0:512], lhsT=kA[:, kblk],
                                     rhs=qA[:, 0:512], start=True, stop=True)
                    nc.tensor.matmul(sc0[:, 512:T], lhsT=kB[:, kblk],
                                     rhs=qB[:, 0:512], start=True, stop=True)
                    nc.tensor.matmul(sc1[:, 0:512], lhsT=kA[:, kblk],
                                     rhs=qA[:, 512:T], start=True, stop=True)
                    nc.tensor.matmul(sc1[:, 512:T], lhsT=kB[:, kblk],
                                     rhs=qB[:, 512:T], start=True, stop=True)
                    # exp: head A on ScalarE, head B on VectorE (bit trick)
                    atA = pata.tile([128, T], BF16, tag="atA", name="atA")
                    nc.scalar.activation(out=atA[:, 0:512], in_=sc0[:, 0:512],
                                         func=AF.Exp, scale=0.125)
                    nc.scalar.activation(out=atA[:, 512:T], in_=sc1[:, 0:512],
                                         func=AF.Exp, scale=0.125)
                    atBi = patb.tile([128, T], I16, tag="atB", name="atB")
                    nc.vector.tensor_scalar(out=atBi[:, 0:512],
                                            in0=sc0[:, 512:T], scalar1=SCH_A,
                                            scalar2=SCH_B, op0=ALU.mult,
                                            op1=ALU.add)
                    nc.vector.tensor_scalar(out=atBi[:, 512:T],
                                            in0=sc1[:, 512:T], scalar1=SCH_A,
                                            scalar2=SCH_B, op0=ALU.mult,
                                            op1=ALU.add)
                    atB = atBi.bitcast(BF16)
                    # attn@V accumulation
                    vr = vp_sb[kb].rearrange("p (h c) -> p h c", c=VW)
                    st, sp = (kb == 0), (kb == SB - 1)
                    nc.tensor.matmul(cxa[:, 0:512], lhsT=vr[:, 2 * j, :],
                                     rhs=atA[:, 0:512], start=st, stop=sp)
                    nc.tensor.matmul(cxb[:, 0:512], lhsT=vr[:, 2 * j + 1, :],
                                     rhs=atB[:, 0:512], start=st, stop=sp)
                    nc.tensor.matmul(cxa[:, 512:T], lhsT=vr[:, 2 * j, :],
                                     rhs=atA[:, 512:T], start=st, stop=sp)
                    nc.tensor.matmul(cxb[:, 512:T], lhsT=vr[:, 2 * j + 1, :],
                                     rhs=atB[:, 512:T], start=st, stop=sp)
                # evacuate ctx + denominators: head A via DVE, head B via
                # ScalarE (the PSUM-capable engines); den rows ride along
                # in the [65, T] staging copies, then DMAs split them out.
                stga = ptmp.tile([VW, T], BF16, tag="stga", name="stga")
                nc.vector.tensor_copy(out=stga, in_=cxa)
                stgb = ptmp.tile([VW, T], BF16, tag="stgb", name="stgb")
                nc.scalar.activation(out=stgb, in_=cxb, func=AF.Copy)
                nc.sync.dma_start(out=ctxT_sb[j][0:64, :], in_=stga[0:64, :])
                nc.sync.dma_start(out=ctxT_sb[j][64:128, :],
                                  in_=stgb[0:64, :])
                nc.gpsimd.dma_start(out=den_dram[2 * j, :],
                                    in_=stga[64:65, :])
                nc.gpsimd.dma_start(out=den_dram[2 * j + 1, :],
                                    in_=stgb[64:65, :])
                # deferred softmax normalization (off critical path):
                # magic-number bf16 reciprocal of broadcast denominators,
                # with the x64 fp8 ctx scale folded into the magic bits.
                dbc = pnm.tile([128, T], BF16, tag="dbc", name="dbc")
                nc.gpsimd.dma_start(
                    out=dbc[0:64, :],
                    in_=bcast_ap(den_dram[2 * j:2 * j + 1, :], 64))
                nc.gpsimd.dma_start(
                    out=dbc[64:128, :],
                    in_=bcast_ap(den_dram[2 * j + 1:2 * j + 2, :], 64))
                rbc = pnm.tile([128, T], I16, tag="rbc", name="rbc")
                nc.vector.tensor_scalar(out=rbc, in0=dbc.bitcast(I16),
                                        scalar1=-1,
                                        scalar2=RCP_MAGIC + (6 << 7),
                                        op0=ALU.mult, op1=ALU.add)
                nc.vector.tensor_mul(out=ctx8_sb[j // 2][:, j % 2, :],
                                     in0=ctxT_sb[j], in1=rbc.bitcast(BF16))

        # ================= FC + residual + layernorm ====================
        with tc.tile_pool(name="fcps", bufs=2, space="PSUM") as pfc, \
             tc.tile_pool(name="lnbc", bufs=1) as plnb, \
             tc.tile_pool(name="xln", bufs=2) as px, \
             tc.tile_pool(name="stat", bufs=4) as pst:
            gamma_bc = plnb.tile([128, D], F32, tag="gamma_bc", name="gamma_bc")
            nc.gpsimd.dma_start(out=gamma_bc, in_=bcast_ap(gamma, 128))
            beta_bc = plnb.tile([128, D], F32, tag="beta_bc", name="beta_bc")
            nc.gpsimd.dma_start(out=beta_bc, in_=bcast_ap(beta, 128))

            for t in range(TB):
                tblk = slice(t * 128, (t + 1) * 128)
                fc = pfc.tile([128, D], F32, tag="fc", name="fc")
                for c0, cn in nsplits(D):
                    for g in range(G):
                        nc.tensor.matmul(
                            fc[:, c0:c0 + cn],
                            lhsT=ctx8_sb[g][:, :, tblk],
                            rhs=wfc_dr[g][:, :, c0:c0 + cn],
                            start=(g == 0), stop=False, perf_mode=DR)
                    # residual: transpose qpT pair blocks via identity
                    # (identity prescaled by LAM to match fp8 scales)
                    for jj in range(c0 // 128, (c0 + cn) // 128):
                        nc.tensor.matmul(
                            fc[:, jj * 128:(jj + 1) * 128],
                            lhsT=qpT_sb[jj][:, tblk], rhs=i_sb,
                            start=False, stop=False)
                    # bfc bias via K=1 ones matmul (marks group end)
                    nc.tensor.matmul(
                        fc[:, c0:c0 + cn], lhsT=ones1,
                        rhs=bfc_sb[0:1, c0:c0 + cn], start=False, stop=True)
                ngr = max(D // 512, 1)
                gsz = min(D, 512)
                stats = pst.tile([128, ngr, 6], F32, tag="stats", name="stats")
                for g in range(ngr):
                    nc.vector.bn_stats(out=stats[:, g, :],
                                       in_=fc[:, g * gsz:(g + 1) * gsz])
                mv = pst.tile([128, 2], F32, tag="mv", name="mv")
                nc.vector.bn_aggr(out=mv, in_=stats)
                rstd = pst.tile([128, 1], F32, tag="rstd", name="rstd")
                nc.scalar.activation(out=rstd, in_=mv[:, 1:2], func=AF.Sqrt,
                                     bias=eps_t, scale=1.0)
                nc.vector.reciprocal(out=rstd, in_=rstd)
                xn = px.tile([128, D], F32, tag="xn", name="xn")
                nc.vector.tensor_scalar(out=xn, in0=fc, scalar1=mv[:, 0:1],
                                        scalar2=rstd, op0=ALU.subtract,
                                        op1=ALU.mult)
                xg = px.tile([128, D], F32, tag="xg", name="xg")
                nc.vector.tensor_mul(out=xg, in0=xn, in1=gamma_bc)
                nc.gpsimd.tensor_add(out=xg, in0=xg, in1=beta_bc)
                nc.sync.dma_start(out=out[tblk, :], in_=xg)

    nc.compile()
    return nc


_B, _S, _D, _H, _DK = 4, 2048, 1024, 16, 64
_T = _S // 2
_NCORES = 8
_BF = ml_dtypes.bfloat16
_F8 = ml_dtypes.float8_e4m3

_nc_cache = [None]


def _get_nc():
    if _nc_cache[0] is None:
        _nc_cache[0] = build(T=_T, S=_S, D=_D, H=_H, DK=_DK, n_cores=_NCORES)
    return _nc_cache[0]


def _f8(x):
    return np.clip(x, -240.0, 240.0).astype(_F8)


def _execute(inputs, trace=False):
    from concourse.bass_utils import run_bass_kernel_spmd

    nc = _get_nc()
    q = np.asarray(inputs["q"], np.float32)
    k = np.asarray(inputs["k"], np.float32)
    v = np.asarray(inputs["v"], np.float32)
    Wq = np.asarray(inputs["Wq"], np.float32).astype(_BF)
    Wk = _f8(np.asarray(inputs["Wk"], np.float32) * 64.0)
    Wv = _f8(np.asarray(inputs["Wv"], np.float32) * 64.0)
    Wfc = _f8(np.asarray(inputs["Wfc"], np.float32) * 64.0)
    fp = {n: np.asarray(inputs[n], np.float32)
          for n in ("bq", "bk", "bv", "gamma", "beta")}
    bfch = (np.asarray(inputs["bfc"], np.float32) * 4096.0).astype(_BF)
    ident = (np.eye(128, dtype=np.float32) * 4096.0).astype(_BF)

    in_maps = []
    for c in range(_NCORES):
        b, half = divmod(c, 2)
        t0 = half * _T
        in_maps.append({
            "qT": np.ascontiguousarray(q[b, t0:t0 + _T].T).astype(_BF),
            "kT": _f8(np.ascontiguousarray(k[b].T)),
            "vT": _f8(np.ascontiguousarray(v[b].T)),
            "Wq": Wq, "Wk": Wk, "Wv": Wv, "Wfc": Wfc,
            "bfch": bfch, "ident": ident, **fp,
        })

    res = run_bass_kernel_spmd(nc, in_maps, core_ids=list(range(_NCORES)),
                               trace=trace)
    out = np.empty((_B, _S, _D), np.float32)
    for c in range(_NCORES):
        b, half = divmod(c, 2)
        out[b, half * _T:(half + 1) * _T] = res.results[c]["out"]
    return out, res.exec_time_ns


def kernel(**inputs) -> np.ndarray:
    out, _ = _execute(inputs, trace=False)
    return out


# revision 11
# speedup vs baseline: 1.3185x; 1.0296x over previous
"""Trainium2 Bass kernel for nn_AttentionLayer (B=4, S=2048, D=1024, H=16).

Self-contained: builds and compiles an SPMD Bass/Tile program once, then
runs it across 8 NeuronCores via run_bass_kernel_spmd.

Sharding (no collectives): core c handles batch b = c // 2 and query-token
half c % 2 (1024 query tokens). Each core receives pre-transposed fp8
activations (x^T slices) plus fp8/bf16 weights, computes its [1024, 1024]
slice of the final layernorm output in fp32, and the host reassembles.

v3 pipeline:
- K/Q^T/V projections and the FC context matmuls run in fp8 (e4m3) with
  DoubleRow perf mode: operands are staged as [128, 2, N] tiles holding
  two 128-row contraction slabs, halving matmul streaming time. Weights
  are host-prescaled by 64 (fp8 range); evacuations rescale by 1/64.
- Attention per head pair: scores^T = Kh @ Qh^T in bf16 with both heads'
  K=64 matmuls packed into one PSUM tile per query chunk — shared WAR
  deps make the scheduler emit them adjacently, so they row-tile onto
  disjoint PE-array halves and run concurrently.
- exp head A on ScalarE (native, bf16 out); head B on VectorE via a
  Schraudolph bit trick (one tensor_scalar fp32->int16 whose bits are
  bf16 exp values; the ~3% sawtooth error cancels in softmax).
- attn@V in bf16 with a per-head ones column producing denominators.
- Softmax normalization deferred: denominators roundtrip DRAM (bf16),
  reciprocal via a magic-number bit trick fused with the x64 fp8 scale,
  producing normalized fp8 context tiles in DoubleRow pair layout.
- FC: fp8 DR ctx matmuls + residual via identity matmuls (transposing
  Q^T tiles, identity prescaled by 4096 to match the fp8 scales) + bfc
  via a K=1 ones matmul; layernorm absorbs the 4096 scale (eps * 4096^2).
"""

import numpy as np
import ml_dtypes


from contextlib import ExitStack

import concourse.bass as bass
import concourse.tile as tile
import concourse.mybir as mybir
from concourse import bacc

F32 = mybir.dt.float32
BF16 = mybir.dt.bfloat16
I16 = mybir.dt.int16
I8 = mybir.dt.int8
F8 = mybir.dt.float8e4
DR = mybir.MatmulPerfMode.DoubleRow
AF = mybir.ActivationFunctionType
ALU = mybir.AluOpType

LOG2E = 1.4426950408889634
# exp(x/8) ~= bf16_bits(int16(x * SCH_A + SCH_B)) (Schraudolph, bf16 top bits)
SCH_A = float((1 << 23) * LOG2E) * 0.125 / 65536.0
SCH_B = (float(127 << 23) - 366393.0) / 65536.0
# same trick to fp8e4m3 bits directly (int8 out): exp(x/8) ~= f8_bits(i8)
SCH_A8 = SCH_A / 16.0
SCH_B8 = (SCH_B - 15360.0) / 16.0
# 1/x ~= bf16_bits(RCP_MAGIC - bf16_bits(x)), x > 0 (max rel err ~5%)
RCP_MAGIC = 0x7EF3
WSCALE = 64.0          # host fp8 weight prescale
LAM = WSCALE * WSCALE  # fc psum scale (ctx*64 @ Wfc*64); LN is scale-invariant


def bcast_ap(ap: bass.AP, parts: int) -> bass.AP:
    """Partition-broadcast a [1, N]-shaped DRAM AP to [parts, N]."""
    return bass.AP(tensor=ap.tensor, offset=ap.offset,
                   ap=[[0, parts]] + list(ap.ap[-1:]))


def nsplits(total, cap=512):
    return [(i, min(cap, total - i)) for i in range(0, total, cap)]


def build(T=1024, S=2048, D=1024, H=16, DK=64, n_cores=8, eps=1e-5,
          trn_type="TRN2", apply_affine=True):
    assert DK == 64 and H % 2 == 0 and D == H * DK
    G = D // 256      # DoubleRow contraction groups (256 rows each)
    EB = D // 128     # e blocks (projection output chunks); == H//2
    TB = T // 128
    SB = S // 128
    PAIRS = H // 2
    VW = 65           # per-head vp stripe: 64 v columns + 1 ones column

    nc = bacc.Bacc(trn_type, target_bir_lowering=False, debug=False,
                   num_devices=n_cores)

    qT = nc.dram_tensor("qT", [D, T], BF16, kind="ExternalInput").ap()
    kT = nc.dram_tensor("kT", [D, S], F8, kind="ExternalInput").ap()
    vT = nc.dram_tensor("vT", [D, S], F8, kind="ExternalInput").ap()
    Wq = nc.dram_tensor("Wq", [D, D], BF16, kind="ExternalInput").ap()
    Wk = nc.dram_tensor("Wk", [D, D], F8, kind="ExternalInput").ap()
    Wv = nc.dram_tensor("Wv", [D, D], F8, kind="ExternalInput").ap()
    Wfc = nc.dram_tensor("Wfc", [D, D], F8, kind="ExternalInput").ap()
    bq = nc.dram_tensor("bq", [D], F32, kind="ExternalInput").ap()
    bk = nc.dram_tensor("bk", [D], F32, kind="ExternalInput").ap()
    bv = nc.dram_tensor("bv", [D], F32, kind="ExternalInput").ap()
    bfch = nc.dram_tensor("bfch", [D], BF16, kind="ExternalInput").ap()
    gamma = nc.dram_tensor("gamma", [D], F32, kind="ExternalInput").ap()
    beta = nc.dram_tensor("beta", [D], F32, kind="ExternalInput").ap()
    ident = nc.dram_tensor("ident", [128, 128], BF16, kind="ExternalInput").ap()
    out = nc.dram_tensor("out", [T, D], F32, kind="ExternalOutput").ap()

    den_dram = nc.dram_tensor("den_scratch", [H, T], BF16).ap()

    def load_dr(pool, src, n, tagp, chunk=None):
        """Load fp8 [D, n] DRAM tensor into G [128, 2, n] DoubleRow tiles."""
        tiles = [pool.tile([128, 2, n], F8, tag=f"{tagp}{g}", name=f"{tagp}{g}")
                 for g in range(G)]
        for c0, cn in nsplits(n, chunk or n):
            for g in range(G):
                t = tiles[g]
                nc.sync.dma_start(out=t[:, 0, c0:c0 + cn],
                                  in_=src[g * 256:g * 256 + 128, c0:c0 + cn])
                nc.sync.dma_start(out=t[:, 1, c0:c0 + cn],
                                  in_=src[g * 256 + 128:g * 256 + 256,
                                          c0:c0 + cn])
        return tiles

    with tile.TileContext(nc) as tc, ExitStack() as ctx:
        pconst = ctx.enter_context(tc.tile_pool(name="const", bufs=1))
        ppers = ctx.enter_context(tc.tile_pool(name="persist", bufs=1))

        # ---- tiny constants -------------------------------------------
        bqT = pconst.tile([128, EB], F32, tag="bqT", name="bqT")
        nc.sync.dma_start(out=bqT, in_=bq.rearrange("(e p) -> p e", p=128))
        bkT = pconst.tile([128, EB], F32, tag="bkT", name="bkT")
        nc.sync.dma_start(out=bkT, in_=bk.rearrange("(e p) -> p e", p=128))
        eps_t = pconst.tile([128, 1], F32, tag="eps", name="eps")
        nc.vector.memset(eps_t, eps * LAM * LAM)
        i_sb = pconst.tile([128, 128], BF16, tag="ident", name="ident")
        nc.sync.dma_start(out=i_sb, in_=ident)
        ones1 = pconst.tile([1, 128], BF16, tag="ones1", name="ones1")
        nc.vector.memset(ones1, 1.0)
        bfc_sb = pconst.tile([1, D], BF16, tag="bfc_sb", name="bfc_sb")
        nc.sync.dma_start(out=bfc_sb, in_=bcast_ap(bfch, 1))

        # ---- persistent tiles -----------------------------------------
        kpT_sb = [ppers.tile([128, S], BF16, tag=f"kpT{e}", name=f"kpT{e}")
                  for e in range(EB)]
        vp_dr = [ppers.tile([128, 2, H * VW], F8, tag=f"vp{s2}",
                            name=f"vp{s2}") for s2 in range(SB // 2)]
        ctxT_sb = [ppers.tile([128, T], BF16, tag=f"ctxT{e}", name=f"ctxT{e}")
                   for e in range(EB)]
        ctx8_sb = [ppers.tile([128, 2, T], F8, tag=f"ctx8_{g}",
                              name=f"ctx8_{g}") for g in range(G)]
        qpT_sb = [ppers.tile([128, T], BF16, tag=f"qpT{j}", name=f"qpT{j}")
                  for j in range(PAIRS)]

        # ================= K projection (e-outer, fp8 DR) ==============
        with tc.tile_pool(name="wk", bufs=1) as pw, \
             tc.tile_pool(name="kx", bufs=1) as pkx, \
             tc.tile_pool(name="qx", bufs=1) as pqx, \
             tc.tile_pool(name="wq", bufs=1) as pwq, \
             tc.tile_pool(name="kps", bufs=2, space="PSUM") as pps, \
             tc.tile_pool(name="qps", bufs=2, space="PSUM") as pqps:
            wk_dr = load_dr(pw, Wk, D, "wk")
            kx_dr = load_dr(pkx, kT, S, "kx", chunk=1024)
            DB = D // 128
            qx_sb = [pqx.tile([128, T], BF16, tag=f"qx{d}", name=f"qx{d}")
                     for d in range(DB)]
            wq_sb = [pwq.tile([128, D], BF16, tag=f"wq{d}", name=f"wq{d}")
                     for d in range(DB)]
            for d in range(DB):
                nc.sync.dma_start(out=qx_sb[d], in_=qT[d * 128:(d + 1) * 128, :])
                nc.sync.dma_start(out=wq_sb[d], in_=Wq[d * 128:(d + 1) * 128, :])
            CK = min(S, 1024)
            for e in range(EB):
                ecol = slice(e * 128, (e + 1) * 128)
                for c0, cn in nsplits(S, CK):
                    ps = pps.tile([128, CK], F32, tag="kpT_ps", name="kpT_ps")
                    for g in range(G):
                        for n0, nn in nsplits(cn):
                            nc.tensor.matmul(
                                ps[:, n0:n0 + nn],
                                lhsT=wk_dr[g][:, :, ecol],
                                rhs=kx_dr[g][:, :, c0 + n0:c0 + n0 + nn],
                                start=(g == 0), stop=(g == G - 1),
                                perf_mode=DR)
                    nc.vector.tensor_scalar(
                        out=kpT_sb[e][:, c0:c0 + cn], in0=ps[:, 0:cn],
                        scalar1=1.0 / WSCALE, scalar2=bkT[:, e:e + 1],
                        op0=ALU.mult, op1=ALU.add)

            # ============= Q^T projection (all pairs, fp8 DR) ==========
            for j in range(PAIRS):
                qps = pqps.tile([128, T], F32, tag="qps", name="qps")
                for d in range(DB):
                    for n0, nn in nsplits(T):
                        nc.tensor.matmul(
                            qps[:, n0:n0 + nn],
                            lhsT=wq_sb[d][:, j * 128:(j + 1) * 128],
                            rhs=qx_sb[d][:, n0:n0 + nn],
                            start=(d == 0), stop=(d == DB - 1))
                nc.vector.tensor_scalar(
                    out=qpT_sb[j], in0=qps, scalar1=bqT[:, j:j + 1],
                    scalar2=None, op0=ALU.add)

        # ================= V projection (natural layout, fp8 DR) ========
        with tc.tile_pool(name="wv", bufs=1) as pw, \
             tc.tile_pool(name="vx", bufs=1) as pvx, \
             tc.tile_pool(name="vbc", bufs=1) as pvbc, \
             tc.tile_pool(name="vps", bufs=3, space="PSUM") as pps:
            bv_bc = pvbc.tile([128, D], F32, tag="bv_bc", name="bv_bc")
            nc.gpsimd.dma_start(out=bv_bc, in_=bcast_ap(bv, 128))
            wv_dr = load_dr(pw, Wv, D, "wv")
            vx_dr = load_dr(pvx, vT, S, "vx")
            for s in range(SB):
                ps = pps.tile([128, D], F32, tag="vp_ps", name="vp_ps")
                for g in range(G):
                    for n0, nn in nsplits(D):
                        nc.tensor.matmul(
                            ps[:, n0:n0 + nn],
                            lhsT=vx_dr[g][:, :, s * 128:(s + 1) * 128],
                            rhs=wv_dr[g][:, :, n0:n0 + nn],
                            start=(g == 0), stop=(g == G - 1), perf_mode=DR)
                vr = vp_dr[s // 2].rearrange("p k (h c) -> p k h c",
                                             c=VW)
                nc.vector.scalar_tensor_tensor(
                    out=vr[:, s % 2, :, 0:64],
                    in0=ps.rearrange("p (h c) -> p h c", c=DK),
                    scalar=1.0 / WSCALE,
                    in1=bv_bc.rearrange("p (h c) -> p h c", c=DK),
                    op0=ALU.mult, op1=ALU.add)
                nc.vector.memset(vr[:, s % 2, :, 64:65], 1.0)

        # ================= attention ====================================
        pwfc = ctx.enter_context(tc.tile_pool(name="wfc", bufs=1))
        wfc_dr = []
        for g in range(G):
            t = pwfc.tile([128, 2, D], F8, tag=f"wfc{g}", name=f"wfc{g}")
            nc.gpsimd.dma_start(out=t[:, 0, :],
                                in_=Wfc[g * 256:g * 256 + 128, :])
            nc.gpsimd.dma_start(out=t[:, 1, :],
                                in_=Wfc[g * 256 + 128:g * 256 + 256, :])
            wfc_dr.append(t)

        with tc.tile_pool(name="scAB0", bufs=1, space="PSUM") as psc0, \
             tc.tile_pool(name="scAB1", bufs=1, space="PSUM") as psc1, \
             tc.tile_pool(name="cxps", bufs=1, space="PSUM") as pcx, \
             tc.tile_pool(name="atA", bufs=2) as pata, \
             tc.tile_pool(name="atB", bufs=2) as patb, \
             tc.tile_pool(name="norm", bufs=2) as pnm, \
             tc.tile_pool(name="ctmp", bufs=2) as ptmp:
            for j in range(PAIRS):
                cxa = pcx.tile([VW, T], F32, tag="cxA", name="cxA")
                cxb = pcx.tile([VW, T], F32, tag="cxB", name="cxB")
                kA = kpT_sb[j][0:64, :]
                kB = kpT_sb[j][64:128, :]
                qA = qpT_sb[j][0:64, :]
                qB = qpT_sb[j][64:128, :]
                for kb in range(SB):
                    kblk = slice(kb * 128, (kb + 1) * 128)
                    # both heads' scores for a query chunk share one PSUM
                    # tile (A in cols 0:512, B in 512:1024) so their WAR
                    # deps coincide -> scheduler emits them adjacently ->
                    # K=64 row-tiles run concurrently on the PE array.
                    sc0 = psc0.tile([128, T], F32, tag="sc0", name="sc0")
                    sc1 = psc1.tile([128, T], F32, tag="sc1", name="sc1")
                    nc.tensor.matmul(sc0[:, 0:512], lhsT=kA[:, kblk],
                                     rhs=qA[:, 0:512], start=True, stop=True)
                    nc.tensor.matmul(sc0[:, 512:T], lhsT=kB[:, kblk],
                                     rhs=qB[:, 0:512], start=True, stop=True)
                    nc.tensor.matmul(sc1[:, 0:512], lhsT=kA[:, kblk],
                                     rhs=qA[:, 512:T], start=True, stop=True)
                    nc.tensor.matmul(sc1[:, 512:T], lhsT=kB[:, kblk],
                                     rhs=qB[:, 512:T], start=True, stop=True)
                    # exp: head A on ScalarE (fp8 out), head B on VectorE
                    # (Schraudolph bit trick straight to fp8e4m3 bits)
                    sl = kb % 2
                    if sl == 0:
                        atA8 = pata.tile([128, 2, T], F8, tag="atA",
                                         name="atA")
                        atB8 = patb.tile([128, 2, T], I8, tag="atB",
                                         name="atB")
                    nc.scalar.activation(out=atA8[:, sl, 0:512],
                                         in_=sc0[:, 0:512],
                                         func=AF.Exp, scale=0.125)
                    nc.scalar.activation(out=atA8[:, sl, 512:T],
                                         in_=sc1[:, 0:512],
                                         func=AF.Exp, scale=0.125)
                    nc.vector.tensor_scalar(out=atB8[:, sl, 0:512],
                                            in0=sc0[:, 512:T], scalar1=SCH_A8,
                                            scalar2=SCH_B8, op0=ALU.mult,
                                            op1=ALU.add)
                    nc.vector.tensor_scalar(out=atB8[:, sl, 512:T],
                                            in0=sc1[:, 512:T], scalar1=SCH_A8,
                                            scalar2=SCH_B8, op0=ALU.mult,
                                            op1=ALU.add)
                    if sl == 1:
                        # attn@V in fp8 DoubleRow: 256-key contraction
                        kb2 = kb // 2
                        vrA = vp_dr[kb2][:, :, 2 * j * VW:2 * j * VW + VW]
                        vrB = vp_dr[kb2][:, :,
                                         (2 * j + 1) * VW:(2 * j + 2) * VW]
                        atB8f = atB8.bitcast(F8)
                        st, sp = (kb2 == 0), (kb2 == SB // 2 - 1)
                        nc.tensor.matmul(cxa[:, 0:512], lhsT=vrA,
                                         rhs=atA8[:, :, 0:512], start=st,
                                         stop=sp, perf_mode=DR)
                        nc.tensor.matmul(cxb[:, 0:512], lhsT=vrB,
                                         rhs=atB8f[:, :, 0:512], start=st,
                                         stop=sp, perf_mode=DR)
                        nc.tensor.matmul(cxa[:, 512:T], lhsT=vrA,
                                         rhs=atA8[:, :, 512:T], start=st,
                                         stop=sp, perf_mode=DR)
                        nc.tensor.matmul(cxb[:, 512:T], lhsT=vrB,
                                         rhs=atB8f[:, :, 512:T], start=st,
                                         stop=sp, perf_mode=DR)
                # evacuate ctx + denominators: head A via DVE, head B via
                # ScalarE (the PSUM-capable engines); den rows ride along
                # in the [65, T] staging copies, then DMAs split them out.
                stga = ptmp.tile([VW, T], BF16, tag="stga", name="stga")
                nc.vector.tensor_copy(out=stga, in_=cxa)
                stgb = ptmp.tile([VW, T], BF16, tag="stgb", name="stgb")
                nc.scalar.activation(out=stgb, in_=cxb, func=AF.Copy)
                nc.sync.dma_start(out=ctxT_sb[j][0:64, :], in_=stga[0:64, :])
                nc.sync.dma_start(out=ctxT_sb[j][64:128, :],
                                  in_=stgb[0:64, :])
                nc.gpsimd.dma_start(out=den_dram[2 * j, :],
                                    in_=stga[64:65, :])
                nc.gpsimd.dma_start(out=den_dram[2 * j + 1, :],
                                    in_=stgb[64:65, :])
                # deferred softmax normalization (off critical path):
                # magic-number bf16 reciprocal of broadcast denominators,
                # with the x64 fp8 ctx scale folded into the magic bits.
                dbc = pnm.tile([128, T], BF16, tag="dbc", name="dbc")
                nc.gpsimd.dma_start(
                    out=dbc[0:64, :],
                    in_=bcast_ap(den_dram[2 * j:2 * j + 1, :], 64))
                nc.gpsimd.dma_start(
                    out=dbc[64:128, :],
                    in_=bcast_ap(den_dram[2 * j + 1:2 * j + 2, :], 64))
                rbc = pnm.tile([128, T], I16, tag="rbc", name="rbc")
                nc.vector.tensor_scalar(out=rbc, in0=dbc.bitcast(I16),
                                        scalar1=-1,
                                        scalar2=RCP_MAGIC + (6 << 7),
                                        op0=ALU.mult, op1=ALU.add)
                nc.vector.tensor_mul(out=ctx8_sb[j // 2][:, j % 2, :],
                                     in0=ctxT_sb[j], in1=rbc.bitcast(BF16))

        # ================= FC + residual + layernorm ====================
        with tc.tile_pool(name="fcps", bufs=2, space="PSUM") as pfc, \
             tc.tile_pool(name="lnbc", bufs=1) as plnb, \
             tc.tile_pool(name="xln", bufs=2) as px, \
             tc.tile_pool(name="stat", bufs=4) as pst:
            if apply_affine:
                gamma_bc = plnb.tile([128, D], F32, tag="gamma_bc",
                                     name="gamma_bc")
                nc.gpsimd.dma_start(out=gamma_bc, in_=bcast_ap(gamma, 128))
                beta_bc = plnb.tile([128, D], F32, tag="beta_bc",
                                    name="beta_bc")
                nc.gpsimd.dma_start(out=beta_bc, in_=bcast_ap(beta, 128))

            for t in range(TB):
                tblk = slice(t * 128, (t + 1) * 128)
                fc = pfc.tile([128, D], F32, tag="fc", name="fc")
                for c0, cn in nsplits(D):
                    for g in range(G):
                        nc.tensor.matmul(
                            fc[:, c0:c0 + cn],
                            lhsT=ctx8_sb[g][:, :, tblk],
                            rhs=wfc_dr[g][:, :, c0:c0 + cn],
                            start=(g == 0), stop=False, perf_mode=DR)
                    # residual: transpose qpT pair blocks via identity
                    # (identity prescaled by LAM to match fp8 scales)
                    for jj in range(c0 // 128, (c0 + cn) // 128):
                        nc.tensor.matmul(
                            fc[:, jj * 128:(jj + 1) * 128],
                            lhsT=qpT_sb[jj][:, tblk], rhs=i_sb,
                            start=False, stop=False)
                    # bfc bias via K=1 ones matmul (marks group end)
                    nc.tensor.matmul(
                        fc[:, c0:c0 + cn], lhsT=ones1,
                        rhs=bfc_sb[0:1, c0:c0 + cn], start=False, stop=True)
                ngr = max(D // 512, 1)
                gsz = min(D, 512)
                stats = pst.tile([128, ngr, 6], F32, tag="stats", name="stats")
                for g in range(ngr):
                    nc.vector.bn_stats(out=stats[:, g, :],
                                       in_=fc[:, g * gsz:(g + 1) * gsz])
                mv = pst.tile([128, 2], F32, tag="mv", name="mv")
                nc.vector.bn_aggr(out=mv, in_=stats)
                rstd = pst.tile([128, 1], F32, tag="rstd", name="rstd")
                nc.scalar.activation(out=rstd, in_=mv[:, 1:2], func=AF.Sqrt,
                                     bias=eps_t, scale=1.0)
                nc.vector.reciprocal(out=rstd, in_=rstd)
                xn = px.tile([128, D], F32, tag="xn", name="xn")
                nc.vector.tensor_scalar(out=xn, in0=fc, scalar1=mv[:, 0:1],
                                        scalar2=rstd, op0=ALU.subtract,
                                        op1=ALU.mult)
                if apply_affine:
                    xg = px.tile([128, D], F32, tag="xg", name="xg")
                    nc.vector.tensor_mul(out=xg, in0=xn, in1=gamma_bc)
                    nc.gpsimd.tensor_add(out=xg, in0=xg, in1=beta_bc)
                else:
                    xg = xn
                nc.sync.dma_start(out=out[tblk, :], in_=xg)

    nc.compile()
    return nc


_B, _S, _D, _H, _DK = 4, 2048, 1024, 16, 64
_T = _S // 2
_NCORES = 8
_BF = ml_dtypes.bfloat16
_F8 = ml_dtypes.float8_e4m3

_nc_cache = {}


def _get_nc(apply_affine):
    if apply_affine not in _nc_cache:
        _nc_cache[apply_affine] = build(T=_T, S=_S, D=_D, H=_H, DK=_DK,
                                        n_cores=_NCORES,
                                        apply_affine=apply_affine)
    return _nc_cache[apply_affine]


def _f8(x):
    return np.clip(x, -240.0, 240.0).astype(_F8)


def _execute(inputs, trace=False):
    from concourse.bass_utils import run_bass_kernel_spmd

    gamma_h = np.asarray(inputs["gamma"], np.float32)
    beta_h = np.asarray(inputs["beta"], np.float32)
    aff = not (np.all(gamma_h == 1.0) and np.all(beta_h == 0.0))
    nc = _get_nc(aff)
    q = np.asarray(inputs["q"], np.float32)
    k = np.asarray(inputs["k"], np.float32)
    v = np.asarray(inputs["v"], np.float32)
    Wq = np.asarray(inputs["Wq"], np.float32).astype(_BF)
    Wk = _f8(np.asarray(inputs["Wk"], np.float32) * 64.0)
    Wv = _f8(np.asarray(inputs["Wv"], np.float32) * 64.0)
    Wfc = _f8(np.asarray(inputs["Wfc"], np.float32) * 64.0)
    fp = {n: np.asarray(inputs[n], np.float32)
          for n in ("bq", "bk", "bv", "gamma", "beta")}
    bfch = (np.asarray(inputs["bfc"], np.float32) * 4096.0).astype(_BF)
    ident = (np.eye(128, dtype=np.float32) * 4096.0).astype(_BF)

    in_maps = []
    for c in range(_NCORES):
        b, half = divmod(c, 2)
        t0 = half * _T
        in_maps.append({
            "qT": np.ascontiguousarray(q[b, t0:t0 + _T].T).astype(_BF),
            "kT": _f8(np.ascontiguousarray(k[b].T)),
            "vT": _f8(np.ascontiguousarray(v[b].T)),
            "Wq": Wq, "Wk": Wk, "Wv": Wv, "Wfc": Wfc,
            "bfch": bfch, "ident": ident, **fp,
        })

    res = run_bass_kernel_spmd(nc, in_maps, core_ids=list(range(_NCORES)),
                               trace=trace)
    out = np.empty((_B, _S, _D), np.float32)
    for c in range(_NCORES):
        b, half = divmod(c, 2)
        out[b, half * _T:(half + 1) * _T] = res.results[c]["out"]
    return out, res.exec_time_ns


def kernel(**inputs) -> np.ndarray:
    out, _ = _execute(inputs, trace=False)
    return out
